# revision 46
# baseline (speedup 1.0000x reference)
import sys
if '/opt/trn_rl_repo' not in sys.path:
    sys.path.insert(0, '/opt/trn_rl_repo')
import numpy as np
import ml_dtypes

import concourse.bass as bass
import concourse.bacc as bacc
import concourse.mybir as mybir
import concourse.tile as tile
from concourse.masks import make_identity

P = 128
N_CORES = 8
LAYERS = 2
RG = [list(range(N_CORES))]
CAP = 240                    # max chunks per gather/stream group
SCALE_T = 64.0               # fp8 table scale (tables hold 64*x)
SCALE_V = 64.0               # fp8 st scale (st holds 64*val)
SCALE_P = SCALE_T * SCALE_V  # psum domain

F32 = mybir.dt.float32
BF16 = mybir.dt.bfloat16
FP8 = mybir.dt.float8e4
I32 = mybir.dt.int32
NP_BF16 = ml_dtypes.bfloat16
NP_FP8 = mybir.dt.np(FP8)


def _cdiv(a, b):
    return -(-a // b)


# ---------------------------------------------------------------------------
# host-side packing
# ---------------------------------------------------------------------------

def pack_edges_v3(rows, cols, vals, nblk_core, col_pos):
    """Sort edges by dest row into 128-row blocks with per-block chunk counts
    (max over cores, so the SPMD program is identical across cores).
    Returns:
      idx    [N_CORES, P, W]      int32  table position per edge slot
      st     [N_CORES, P, W*128]  fp8    one-hot(row%128) * (64*val)
      cb     [nblk_core]          chunks per block
      groups list of (b0, b1, c0, nch) covering blocks with sum(cb) <= CAP
    """
    nblk_total = N_CORES * nblk_core
    order = np.argsort(rows, kind='stable')
    r = rows[order].astype(np.int64)
    c = col_pos(cols[order].astype(np.int64))
    v = vals[order].astype(np.float32)
    blk = r >> 7
    counts = np.bincount(blk, minlength=nblk_total).reshape(N_CORES, nblk_core)
    cb = _cdiv(counts.max(axis=0), P)            # [nblk_core]
    off = np.zeros(nblk_core + 1, np.int64)
    np.cumsum(cb, out=off[1:])
    W = int(off[-1])

    eoffs = np.zeros(nblk_total + 1, np.int64)
    np.cumsum(counts.ravel(), out=eoffs[1:])
    rank = np.arange(len(r), dtype=np.int64) - eoffs[blk]
    j = rank >> 7
    lane = rank & 127
    core = blk // nblk_core
    lblk = blk % nblk_core
    colpos = off[lblk] + j
    idx = np.zeros((N_CORES, P, W), np.int32)
    st = np.zeros((N_CORES, P, W, P), NP_FP8)
    idx[core, lane, colpos] = c
    st[core, lane, colpos, r & 127] = (v * SCALE_V).astype(NP_FP8)

    groups = []
    b0 = 0
    while b0 < nblk_core:
        b1 = b0
        nch = 0
        while b1 < nblk_core and nch + cb[b1] <= CAP:
            nch += int(cb[b1])
            b1 += 1
        assert b1 > b0
        if nch > 0:
            groups.append((b0, b1, int(off[b0]), nch))
        b0 = b1
    return idx, st.reshape(N_CORES, P, W * P), [int(x) for x in cb], groups, W


def build_head_routing(memb, svc, mash, RWU, RWM, RW, BC):
    def ui_pos(g):
        own = g // RWU
        return own * RW + (g - own * RWU)

    def mm_pos(g):
        own = g // RWM
        return own * RW + RWU + (g - own * RWM)

    keys_mem = ui_pos(memb.astype(np.int64))
    keys_svc = ui_pos(svc.astype(np.int64))
    keys_mash = mm_pos(mash.astype(np.int64))

    uniq = []
    for k in range(N_CORES):
        ak = np.concatenate([keys_mem[k * BC:(k + 1) * BC].ravel(),
                             keys_svc[k * BC:(k + 1) * BC],
                             keys_mash[k * BC:(k + 1) * BC]])
        owner = ak // RW
        uniq.append([np.unique(ak[owner == j]) for j in range(N_CORES)])
    PM = max(len(u) for row in uniq for u in row)
    PM = _cdiv(PM, 16) * 16

    send_idx = np.zeros((N_CORES, N_CORES * PM), np.int32)
    for j in range(N_CORES):
        for k in range(N_CORES):
            u = uniq[k][j]
            send_idx[j, k * PM:k * PM + len(u)] = (u % RW).astype(np.int32)

    def remap(keys):
        out = np.empty(keys.shape, np.int32)
        for k in range(N_CORES):
            kk = keys[k * BC:(k + 1) * BC]
            oo = np.empty(kk.shape, np.int64)
            owner = kk // RW
            for j in range(N_CORES):
                m = owner == j
                oo[m] = j * PM + np.searchsorted(uniq[k][j], kk[m])
            out[k * BC:(k + 1) * BC] = oo
        return out

    return PM, send_idx, remap(keys_mem), remap(keys_svc), remap(keys_mash)


# ---------------------------------------------------------------------------
# bass program builder
# ---------------------------------------------------------------------------

def build_program(cfg):
    D = cfg['D']
    NBU = cfg['NBU']; NBM = cfg['NBM']
    WU = cfg['WU']; WM = cfg['WM']
    CBU = cfg['CBU']; CBM = cfg['CBM']
    GRU = cfg['GRU']; GRM = cfg['GRM']
    BT = cfg['BT']; L = cfg['L']; PM = cfg['PM']

    RWU = NBU * P
    RWM = NBM * P
    RW = RWU + RWM
    KS = (N_CORES * PM) // P
    CAPB = max(b1 - b0 for (b0, b1, _, _) in GRU + GRM)

    nc = bacc.Bacc("TRN2", target_bir_lowering=False, debug=False,
                   enable_asserts=False, num_devices=N_CORES)

    def din(name, shape, dt):
        return nc.dram_tensor(name, shape, dt, kind="ExternalInput").ap()

    x0s = din("x0s", [RWU, D], F32)             # 4096*x ui shard
    m0s = din("m0s", [RWM, D], F32)             # 4096*x mashup shard
    tbl0 = din("tbl0", [N_CORES * RW, D], FP8)  # 64*x interleaved, replicated
    ui_idx = din("ui_idx", [P, WU], I32)
    ui_st = din("ui_st", [P, WU * D], FP8)
    mm_idx = din("mm_idx", [P, WM], I32)
    mm_st = din("mm_st", [P, WM * D], FP8)
    dinv = din("dinv", [P, NBM], F32)
    send_idx = din("send_idx", [P, KS], I32)
    memb_idx = din("memb_idx", [BT * P, L], I32)
    svc_idx = din("svc_idx", [BT * P, 1], I32)
    mash_idx = din("mash_idx", [BT * P, 1], I32)
    mask_lb = din("mask_lb", [BT, L * P], BF16)
    aw1 = din("aw1", [2 * D, 16], F32)
    ab1 = din("ab1", [16], F32)
    aw2 = din("aw2", [16, 1], F32)
    ab2 = din("ab2", [1], F32)
    pw1 = din("pw1", [3 * D, 8], F32)
    pb1 = din("pb1", [8], F32)
    pw2 = din("pw2", [8, 1], F32)
    pb2 = din("pb2", [1], F32)

    y = nc.dram_tensor("y", [BT * P, 1], F32, kind="ExternalOutput").ap()

    with tile.TileContext(nc) as tc:
        with tc.tile_pool(name="dram", bufs=1, space="DRAM") as dram, \
             tc.tile_pool(name="res", bufs=1) as res:
            h1m_in = dram.tile([RW, D], FP8)
            h1m_full = dram.tile([N_CORES * RW, D], FP8, addr_space="Shared")
            uifm_sh = dram.tile([RW, D], BF16)
            a2a_in = dram.tile([N_CORES * PM, D], BF16)
            a2a_out = dram.tile([N_CORES * PM, D], BF16)

            uiidx_t = res.tile([P, WU], I32)
            nc.sync.dma_start(out=uiidx_t[:], in_=ui_idx[:])
            mmidx_t = res.tile([P, WM], I32)
            nc.sync.dma_start(out=mmidx_t[:], in_=mm_idx[:])
            dinv_t = res.tile([P, NBM], F32)
            nc.sync.dma_start(out=dinv_t[:], in_=dinv[:])
            sidx_t = res.tile([P, KS], I32)
            nc.sync.dma_start(out=sidx_t[:], in_=send_idx[:])

            def spmm_layer(sp, sps, pp, tbl, groups, cb, off_of, idx_t,
                           st_dram, post, stage_dt, stage_out, row_off):
                """off_of[b] = chunk offset of block b (host-computed)."""
                if True:
                    for (b0, b1, c0, nch) in groups:
                        st_t = sps.tile([P, CAP * D], FP8, tag="st")
                        nc.sync.dma_start(
                            out=st_t[:, :nch * D],
                            in_=st_dram[:, c0 * D:(c0 + nch) * D])
                        gt = sp.tile([P, CAP * D], FP8, tag="gt")
                        nc.gpsimd.indirect_dma_start(
                            out=gt[:, :nch * D],
                            out_offset=None,
                            in_=tbl[:],
                            in_offset=bass.IndirectOffsetOnAxis(
                                ap=idx_t[:, c0:c0 + nch], axis=0))
                        stage = sp.tile([P, CAPB * D], stage_dt, tag="stage")
                        for b in range(b0, b1):
                            nb = cb[b]
                            if nb == 0:
                                continue
                            pm = pp.tile([P, D], F32, tag="pm")
                            for j in range(nb):
                                o = (off_of[b] - c0 + j) * D
                                nc.tensor.matmul(
                                    out=pm[:], lhsT=st_t[:, o:o + D],
                                    rhs=gt[:, o:o + D],
                                    start=(j == 0), stop=(j == nb - 1))
                            post(b, b - b0, pm, stage)
                        nc.sync.dma_start(
                            out=stage_out[row_off + b0 * P:row_off + b1 * P, :]
                                .rearrange("(b p) d -> p b d", p=P),
                            in_=stage[:, :(b1 - b0) * D]
                                .rearrange("p (b d) -> p b d", d=D))

            off_u = np.zeros(NBU + 1, np.int64)
            np.cumsum(CBU, out=off_u[1:])
            off_m = np.zeros(NBM + 1, np.int64)
            np.cumsum(CBM, out=off_m[1:])

            with tc.tile_pool(name="acc", bufs=1) as accp, \
                 tc.tile_pool(name="sp_sb", bufs=2) as sp, \
                 tc.tile_pool(name="sp_st", bufs=2) as sps, \
                 tc.tile_pool(name="sp_ps", bufs=4, space="PSUM") as pp, \
                 tc.tile_pool(name="mtp", bufs=4) as mtp:
                acc_u = accp.tile([P, NBU * D], F32)
                acc_m = accp.tile([P, NBM * D], F32)
                nc.sync.dma_start(
                    out=acc_u[:].rearrange("p (b d) -> p b d", d=D),
                    in_=x0s.rearrange("(b p) d -> p b d", p=P))
                nc.sync.dma_start(
                    out=acc_m[:].rearrange("p (b d) -> p b d", d=D),
                    in_=m0s.rearrange("(b p) d -> p b d", p=P))

                # ----------------- layer 1 ----------------------------------
                def ui_l1_post(b, bb, pm, stage):
                    nc.scalar.activation(out=stage[:, bb * D:(bb + 1) * D],
                                         in_=pm[:],
                                         func=mybir.ActivationFunctionType.Copy,
                                         scale=1.0 / SCALE_V)
                    nc.vector.tensor_tensor(
                        out=acc_u[:, b * D:(b + 1) * D],
                        in0=acc_u[:, b * D:(b + 1) * D], in1=pm[:],
                        op=mybir.AluOpType.add)

                def mm_l1_post(b, bb, pm, stage):
                    tmp = mtp.tile([P, D], F32, tag="mtmp")
                    nc.vector.tensor_scalar(
                        out=tmp[:], in0=pm[:], scalar1=dinv_t[:, b:b + 1],
                        scalar2=None, op0=mybir.AluOpType.mult)
                    nc.scalar.activation(
                        out=stage[:, bb * D:(bb + 1) * D], in_=tmp[:],
                        func=mybir.ActivationFunctionType.Copy,
                        scale=1.0 / SCALE_V)
                    nc.vector.tensor_tensor(
                        out=acc_m[:, b * D:(b + 1) * D],
                        in0=acc_m[:, b * D:(b + 1) * D], in1=tmp[:],
                        op=mybir.AluOpType.add)

                spmm_layer(sp, sps, pp, tbl0, GRU, CBU, off_u, uiidx_t,
                           ui_st, ui_l1_post, FP8, h1m_in, 0)
                spmm_layer(sp, sps, pp, tbl0, GRM, CBM, off_m, mmidx_t,
                           mm_st, mm_l1_post, FP8, h1m_in, RWU)

                nc.gpsimd.collective_compute(
                    "AllGather", mybir.AluOpType.bypass, replica_groups=RG,
                    ins=[h1m_in[:]], outs=[h1m_full[:]])

                # ----------------- layer 2 ----------------------------------
                fin_s = 1.0 / ((LAYERS + 1) * SCALE_P)

                def ui_l2_post(b, bb, pm, stage):
                    tmp = mtp.tile([P, D], F32, tag="mtmp")
                    nc.vector.tensor_tensor(
                        out=tmp[:], in0=acc_u[:, b * D:(b + 1) * D],
                        in1=pm[:], op=mybir.AluOpType.add)
                    nc.scalar.activation(
                        out=stage[:, bb * D:(bb + 1) * D], in_=tmp[:],
                        func=mybir.ActivationFunctionType.Copy,
                        scale=fin_s)

                spmm_layer(sp, sps, pp, h1m_full, GRU, CBU, off_u, uiidx_t,
                           ui_st, ui_l2_post, BF16, uifm_sh, 0)

                def mm_l2_post(b, bb, pm, stage):
                    tmp = mtp.tile([P, D], F32, tag="mtmp")
                    nc.vector.tensor_scalar(
                        out=tmp[:], in0=pm[:], scalar1=dinv_t[:, b:b + 1],
                        scalar2=None, op0=mybir.AluOpType.mult)
                    nc.vector.tensor_tensor(
                        out=tmp[:], in0=acc_m[:, b * D:(b + 1) * D],
                        in1=tmp[:], op=mybir.AluOpType.add)
                    nc.scalar.activation(
                        out=stage[:, bb * D:(bb + 1) * D], in_=tmp[:],
                        func=mybir.ActivationFunctionType.Copy,
                        scale=fin_s)

                spmm_layer(sp, sps, pp, h1m_full, GRM, CBM, off_m, mmidx_t,
                           mm_st, mm_l2_post, BF16, uifm_sh, RWU)

            # ----------------- head A2A -------------------------------------
            with tc.tile_pool(name="snd", bufs=1) as snd:
                sgt = snd.tile([P, KS * D], BF16)
                nc.gpsimd.indirect_dma_start(
                    out=sgt[:], out_offset=None, in_=uifm_sh[:],
                    in_offset=bass.IndirectOffsetOnAxis(ap=sidx_t[:], axis=0))
                nc.sync.dma_start(
                    out=a2a_in[:].rearrange("(k p) d -> p k d", p=P),
                    in_=sgt[:].rearrange("p (k d) -> p k d", d=D))
                nc.gpsimd.collective_compute(
                    "AllToAll", mybir.AluOpType.bypass, replica_groups=RG,
                    ins=[a2a_in[:]], outs=[a2a_out[:]])

            # ----------------- head -----------------------------------------
            with tc.tile_pool(name="hd", bufs=1) as hd, \
                 tc.tile_pool(name="hd2", bufs=2) as hd2, \
                 tc.tile_pool(name="hd_ps", bufs=2, space="PSUM") as hps, \
                 tc.tile_pool(name="hd_psb", bufs=2, space="PSUM") as hpsb, \
                 tc.tile_pool(name="hd_ps2", bufs=2, space="PSUM") as hps2:
                identb = hd.tile([P, P], BF16)
                make_identity(nc, identb[:])
                ident = hd.tile([P, P], F32)
                make_identity(nc, ident[:])
                ones_t = hd.tile([1, P], BF16)
                nc.vector.memset(ones_t[:], 1.0)
                neg_t = hd.tile([1, P], BF16)
                nc.vector.memset(neg_t[:], -1e9)
                w1m_t = hd.tile([P, 16], F32)
                nc.sync.dma_start(out=w1m_t[:], in_=aw1[0:D, :])
                w1mb_t = hd.tile([P, 16], BF16)
                nc.vector.tensor_copy(out=w1mb_t[:], in_=w1m_t[:])
                w1s_t = hd.tile([P, 16], F32)
                nc.sync.dma_start(out=w1s_t[:], in_=aw1[D:2 * D, :])
                w2_t = hd.tile([16, 1], F32)
                nc.sync.dma_start(out=w2_t[:], in_=aw2[:])
                b1_t = hd.tile([16, 1], F32)
                nc.sync.dma_start(out=b1_t[:], in_=ab1.unsqueeze(1))
                b2_t = hd.tile([1, 1], F32)
                nc.sync.dma_start(out=b2_t[:], in_=ab2.unsqueeze(1))
                b2r_t = hd.tile([P, 1], F32)
                nc.gpsimd.partition_broadcast(b2r_t[:], b2_t[:])
                pw1_t = hd.tile([P, 3 * 8], F32)
                nc.sync.dma_start(
                    out=pw1_t[:].rearrange("p (c h) -> p c h", h=8),
                    in_=pw1.rearrange("(c p) h -> p c h", p=P))
                pb1_t = hd.tile([8, 1], F32)
                nc.sync.dma_start(out=pb1_t[:], in_=pb1.unsqueeze(1))
                pw2_t = hd.tile([8, 1], F32)
                nc.sync.dma_start(out=pw2_t[:], in_=pw2[:])
                pb2_t = hd.tile([1, 1], F32)
                nc.sync.dma_start(out=pb2_t[:], in_=pb2.unsqueeze(1))

                NLB = L * P
                NCK = NLB // 512

                for t in range(BT):
                    midx_t = hd2.tile([P, L], I32, tag="midx")
                    nc.sync.dma_start(out=midx_t[:],
                                      in_=memb_idx[t * P:(t + 1) * P, :])
                    me_bf = hd2.tile([P, L * D], BF16, tag="mebf")
                    nc.gpsimd.indirect_dma_start(
                        out=me_bf[:], out_offset=None, in_=a2a_out[:],
                        in_offset=bass.IndirectOffsetOnAxis(ap=midx_t[:], axis=0))
                    et_t = hd2.tile([P, L * D], BF16, tag="et")
                    for l in range(L):
                        ptr = hpsb.tile([P, P], BF16, tag="ptr")
                        nc.tensor.transpose(out=ptr[:],
                                            in_=me_bf[:, l * D:(l + 1) * D],
                                            identity=identb[:])
                        nc.vector.tensor_copy(out=et_t[:, l * P:(l + 1) * P],
                                              in_=ptr[:])
                    svx_t = hd2.tile([P, 1], I32, tag="svx")
                    nc.sync.dma_start(out=svx_t[:],
                                      in_=svc_idx[t * P:(t + 1) * P, :])
                    sv_bf = hd2.tile([P, D], BF16, tag="svbf")
                    nc.gpsimd.indirect_dma_start(
                        out=sv_bf[:], out_offset=None, in_=a2a_out[:],
                        in_offset=bass.IndirectOffsetOnAxis(ap=svx_t[:], axis=0))
                    sv_t = hd2.tile([P, D], F32, tag="sv")
                    nc.vector.tensor_copy(out=sv_t[:], in_=sv_bf[:])
                    ptr = hps.tile([P, P], F32, tag="ptrf")
                    nc.tensor.transpose(out=ptr[:], in_=sv_t[:], identity=ident[:])
                    svcT_t = hd2.tile([P, P], F32, tag="svcT")
                    nc.vector.tensor_copy(out=svcT_t[:], in_=ptr[:])
                    xidx_t = hd2.tile([P, 1], I32, tag="xidx")
                    nc.sync.dma_start(out=xidx_t[:],
                                      in_=mash_idx[t * P:(t + 1) * P, :])
                    ma_bf = hd2.tile([P, D], BF16, tag="mabf")
                    nc.gpsimd.indirect_dma_start(
                        out=ma_bf[:], out_offset=None, in_=a2a_out[:],
                        in_offset=bass.IndirectOffsetOnAxis(ap=xidx_t[:], axis=0))
                    ma_t = hd2.tile([P, D], F32, tag="ma")
                    nc.vector.tensor_copy(out=ma_t[:], in_=ma_bf[:])
                    ptr = hps.tile([P, P], F32, tag="ptrf")
                    nc.tensor.transpose(out=ptr[:], in_=ma_t[:], identity=ident[:])
                    maT_t = hd2.tile([P, P], F32, tag="maT")
                    nc.vector.tensor_copy(out=maT_t[:], in_=ptr[:])

                    psv = hps2.tile([16, P], F32, tag="ps_small")
                    nc.tensor.matmul(out=psv[:], lhsT=w1s_t[:], rhs=svcT_t[:],
                                     start=True, stop=True)
                    svterm_t = hd2.tile([16, P], F32, tag="svterm")
                    nc.vector.tensor_copy(out=svterm_t[:], in_=psv[:])

                    hdn_t = hd2.tile([16, NLB], F32, tag="hdn")
                    lpc = 512 // P
                    for n in range(NCK):
                        pmt = hps2.tile([16, 512], F32, tag="ps_small")
                        nc.tensor.matmul(out=pmt[:], lhsT=w1mb_t[:],
                                         rhs=et_t[:, n * 512:(n + 1) * 512],
                                         start=True, stop=True)
                        tt = hd2.tile([16, 512], F32, tag="tt16")
                        nc.vector.tensor_tensor(
                            out=tt[:].rearrange("h (l b) -> h l b", b=P),
                            in0=pmt[:].rearrange("h (l b) -> h l b", b=P),
                            in1=svterm_t[:].unsqueeze(1).to_broadcast([16, lpc, P]),
                            op=mybir.AluOpType.add)
                        nc.scalar.activation(
                            out=hdn_t[:, n * 512:(n + 1) * 512], in_=tt[:],
                            func=mybir.ActivationFunctionType.Relu,
                            bias=b1_t[:], scale=1.0)

                    sc_t = hd2.tile([1, NLB], BF16, tag="sc")
                    for n in range(NCK):
                        pst = hps2.tile([1, 512], F32, tag="ps_small")
                        nc.tensor.matmul(out=pst[:], lhsT=w2_t[:],
                                         rhs=hdn_t[:, n * 512:(n + 1) * 512],
                                         start=True, stop=True)
                        nc.vector.tensor_copy(out=sc_t[:, n * 512:(n + 1) * 512],
                                              in_=pst[:])
                    mk_t = hd2.tile([1, NLB], BF16, tag="mk")
                    nc.sync.dma_start(out=mk_t[:], in_=mask_lb[t:t + 1, :])

                    ew_t = hd2.tile([P, NLB], F32, tag="ew")
                    for n in range(NCK):
                        prt = hps.tile([P, 512], F32, tag="prt")
                        nc.tensor.matmul(out=prt[:], lhsT=ones_t[:],
                                         rhs=sc_t[:, n * 512:(n + 1) * 512],
                                         start=True, stop=False)
                        nc.tensor.matmul(out=prt[:], lhsT=neg_t[:],
                                         rhs=mk_t[:, n * 512:(n + 1) * 512],
                                         start=False, stop=True)
                        nc.scalar.activation(
                            out=ew_t[:, n * 512:(n + 1) * 512], in_=prt[:],
                            func=mybir.ActivationFunctionType.Exp,
                            bias=b2r_t[:], scale=1.0)

                    den_t = hd2.tile([P, P], F32, tag="den")
                    nc.vector.tensor_reduce(
                        out=den_t[:],
                        in_=ew_t[:].rearrange("p (l b) -> p b l", b=P),
                        axis=mybir.AxisListType.X, op=mybir.AluOpType.add)
                    rden_t = hd2.tile([P, P], F32, tag="rden")
                    nc.vector.reciprocal(rden_t[:], den_t[:])
                    nc.vector.tensor_tensor(out=ew_t[:], in0=et_t[:], in1=ew_t[:],
                                            op=mybir.AluOpType.mult)
                    gatt_t = hd2.tile([P, P], F32, tag="gatt")
                    nc.vector.tensor_reduce(
                        out=gatt_t[:],
                        in_=ew_t[:].rearrange("p (l b) -> p b l", b=P),
                        axis=mybir.AxisListType.X, op=mybir.AluOpType.add)
                    nc.vector.tensor_tensor(out=gatt_t[:], in0=gatt_t[:],
                                            in1=rden_t[:],
                                            op=mybir.AluOpType.mult)
                    nc.vector.tensor_tensor(out=maT_t[:], in0=gatt_t[:],
                                            in1=maT_t[:], op=mybir.AluOpType.add)
                    elem_t = hd2.tile([P, P], F32, tag="elem")
                    nc.vector.tensor_tensor(out=elem_t[:], in0=maT_t[:],
                                            in1=svcT_t[:],
                                            op=mybir.AluOpType.mult)
                    ppd = hps2.tile([8, P], F32, tag="ps_small")
                    for c, rhs in enumerate((elem_t, maT_t, svcT_t)):
                        nc.tensor.matmul(out=ppd[:], lhsT=pw1_t[:, c * 8:(c + 1) * 8],
                                         rhs=rhs[:], start=(c == 0), stop=(c == 2))
                    hp_t = hd2.tile([8, P], F32, tag="hp")
                    nc.scalar.activation(out=hp_t[:], in_=ppd[:],
                                         func=mybir.ActivationFunctionType.Relu,
                                         bias=pb1_t[:], scale=1.0)
                    pyt = hps2.tile([1, P], F32, tag="ps_small")
                    nc.tensor.matmul(out=pyt[:], lhsT=pw2_t[:], rhs=hp_t[:],
                                     start=True, stop=True)
                    y_t = hd2.tile([1, P], F32, tag="yt")
                    nc.scalar.activation(out=y_t[:], in_=pyt[:],
                                         func=mybir.ActivationFunctionType.Sigmoid,
                                         bias=pb2_t[:], scale=1.0)
                    nc.sync.dma_start(out=y[t * P:(t + 1) * P, :], in_=y_t[:])

    nc.compile()
    return nc


# ---------------------------------------------------------------------------
# host orchestration
# ---------------------------------------------------------------------------

def prepare(inputs, bf16=True, gg=None):
    NU, D = inputs['user_tbl'].shape
    NS = inputs['service_tbl'].shape[0]
    NM = inputs['mashup_tbl'].shape[0]
    B, L = inputs['member_masked'].shape

    NBU = _cdiv(NU + NS, N_CORES * P)
    NBM = _cdiv(NM, N_CORES * P)
    RWU, RWM = NBU * P, NBM * P
    RW = RWU + RWM
    BT = B // (N_CORES * P)
    BC = BT * P

    def ui_pos(c):
        own = c // RWU
        return own * RW + (c - own * RWU)

    def mm_pos(c):
        own = c // RWM
        return own * RW + RWU + (c - own * RWM)

    ui_idx, ui_st, CBU, GRU, WU = pack_edges_v3(
        np.asarray(inputs['adj_rows']), np.asarray(inputs['adj_cols']),
        np.asarray(inputs['adj_vals'], np.float32), NBU, ui_pos)
    mm_idx, mm_st, CBM, GRM, WM = pack_edges_v3(
        np.asarray(inputs['A_rows']), np.asarray(inputs['A_cols']),
        np.asarray(inputs['A_vals'], np.float32), NBM, mm_pos)

    x0 = np.zeros((N_CORES * RWU, D), np.float32)
    x0[:NU] = inputs['user_tbl']
    x0[NU:NU + NS] = inputs['service_tbl']
    m0 = np.zeros((N_CORES * RWM, D), np.float32)
    m0[:NM] = inputs['mashup_tbl']

    tbl0 = np.empty((N_CORES * RW, D), NP_FP8)
    for c in range(N_CORES):
        tbl0[c * RW:c * RW + RWU] = (x0[c * RWU:(c + 1) * RWU] *
                                     SCALE_T).astype(NP_FP8)
        tbl0[c * RW + RWU:(c + 1) * RW] = (m0[c * RWM:(c + 1) * RWM] *
                                           SCALE_T).astype(NP_FP8)

    dv = np.zeros(N_CORES * RWM, np.float32)
    dv[:NM] = inputs['d_inv']
    dv = dv.reshape(N_CORES, NBM, P).transpose(0, 2, 1).copy()

    PMx, send_idx, mem_r, svc_r, mash_r = build_head_routing(
        np.asarray(inputs['member_masked'], np.int64),
        np.asarray(inputs['service_inputs'], np.int64) + NU,
        np.asarray(inputs['mashup_inputs'], np.int64),
        RWU, RWM, RW, BC)
    KS = (N_CORES * PMx) // P

    mask = np.asarray(inputs['mask'], np.float32).reshape(N_CORES, BT, P, L)
    mask_lb = mask.transpose(0, 1, 3, 2).reshape(N_CORES, BT, L * P)
    mask_lb = mask_lb.astype(NP_BF16)

    cfg = dict(D=D, L=L, NBU=NBU, NBM=NBM, BT=BT, PM=PMx,
               WU=WU, WM=WM, CBU=tuple(CBU), CBM=tuple(CBM),
               GRU=tuple(GRU), GRM=tuple(GRM))

    in_maps = []
    for k in range(N_CORES):
        in_maps.append({
            'x0s': x0[k * RWU:(k + 1) * RWU] * SCALE_P,
            'm0s': m0[k * RWM:(k + 1) * RWM] * SCALE_P,
            'tbl0': tbl0,
            'ui_idx': ui_idx[k], 'ui_st': ui_st[k],
            'mm_idx': mm_idx[k], 'mm_st': mm_st[k],
            'dinv': dv[k],
            'send_idx': send_idx[k].reshape(KS, P).T.copy(),
            'memb_idx': mem_r[k * BC:(k + 1) * BC],
            'svc_idx': svc_r[k * BC:(k + 1) * BC].reshape(BC, 1),
            'mash_idx': mash_r[k * BC:(k + 1) * BC].reshape(BC, 1),
            'mask_lb': mask_lb[k],
            'aw1': np.asarray(inputs['att_w1'], np.float32),
            'ab1': np.asarray(inputs['att_b1'], np.float32),
            'aw2': np.asarray(inputs['att_w2'], np.float32),
            'ab2': np.asarray(inputs['att_b2'], np.float32),
            'pw1': np.asarray(inputs['pred_w1'], np.float32),
            'pb1': np.asarray(inputs['pred_b1'], np.float32),
            'pw2': np.asarray(inputs['pred_w2'], np.float32),
            'pb2': np.asarray(inputs['pred_b2'], np.float32),
        })
    return cfg, in_maps


_CACHE = {}


def _cfg_key(cfg):
    return tuple(sorted((k, v) for k, v in cfg.items()))


def run(inputs, bf16=True, gg=None, trace=False, tmpdir=None, **tkw):
    from concourse.bass_utils import run_bass_kernel_spmd
    cfg, in_maps = prepare(inputs)
    key = _cfg_key(cfg)
    if key not in _CACHE:
        _CACHE[key] = build_program(cfg)
    nc = _CACHE[key]
    res = run_bass_kernel_spmd(nc, in_maps, core_ids=list(range(N_CORES)),
                               trace=trace, tmpdir=tmpdir, **tkw)
    yy = np.concatenate([r['y'] for r in res.results], axis=0)
    return yy, res


def make_timed_runner(nc, in_maps):
    """Build the sharded PJRT executable with inputs staged on device once."""
    import jax
    from jax.sharding import Mesh, PartitionSpec, NamedSharding
    from jax.experimental.shard_map import shard_map
    from concourse import bass2jax
    from concourse.bass2jax import _bass_exec_p, install_neuronx_cc_hook

    install_neuronx_cc_hook()
    nc_ = nc
    pname = nc_.partition_id_tensor.name if nc_.partition_id_tensor else None
    in_names, out_names, out_avals, zero_outs = [], [], [], []
    for alloc in nc_.m.functions[0].allocations:
        if not isinstance(alloc, mybir.MemoryLocationSet):
            continue
        name = alloc.memorylocations[0].name
        if alloc.kind == "ExternalInput":
            if name != pname:
                in_names.append(name)
        elif alloc.kind == "ExternalOutput":
            out_names.append(name)
            shape = tuple(alloc.tensor_shape)
            dtype = mybir.dt.np(alloc.dtype)
            out_avals.append(jax.core.ShapedArray(shape, dtype))
            zero_outs.append(np.zeros(shape, dtype))
    n_params = len(in_names)
    n_outs = len(out_avals)
    all_names = in_names + out_names
    if pname is not None:
        all_names = all_names + [pname]

    def _body(*args):
        operands = list(args)
        if pname is not None:
            operands.append(bass2jax.partition_id_tensor())
        outs = _bass_exec_p.bind(
            *operands, out_avals=tuple(out_avals), in_names=tuple(all_names),
            out_names=tuple(out_names), lowering_input_output_aliases=(),
            sim_require_finite=True, sim_require_nnan=True, nc=nc_)
        return tuple(outs)

    devices = jax.devices()[:N_CORES]
    mesh = Mesh(np.asarray(devices), ("core",))
    in_specs = (PartitionSpec("core"),) * (n_params + n_outs)
    out_specs = (PartitionSpec("core"),) * n_outs
    donate = tuple(range(n_params, n_params + n_outs))
    sharded = jax.jit(
        shard_map(_body, mesh=mesh, in_specs=in_specs, out_specs=out_specs,
                  check_rep=False),
        donate_argnums=donate, keep_unused=True)
    sh = NamedSharding(mesh, PartitionSpec("core"))
    dev_in = [
        jax.device_put(
            np.concatenate([np.asarray(in_maps[c][nm]) for c in range(N_CORES)],
                           axis=0), sh)
        for nm in in_names]

    def make_zeros():
        return [
            jax.device_put(np.zeros((N_CORES * z.shape[0], *z.shape[1:]), z.dtype), sh)
            for z in zero_outs]

    def call(block=True, zeros=None):
        if zeros is None:
            zeros = make_zeros()
        outs = sharded(*dev_in, *zeros)
        if block:
            jax.block_until_ready(outs)
        return outs

    call.make_zeros = make_zeros
    return call, out_names


def kernel(**inputs) -> np.ndarray:
    yy, _ = run(inputs)
    return yy.astype(np.float32)
